# revision 54
# baseline (speedup 1.0000x reference)
"""Multi-head attention (B=4, S=2048, D=1024, H=16) on 8 trn2 NeuronCores.

Sharding: 8 cores = 4 batches x 2 head-groups. Core c handles batch c//2 and
heads [8g, 8g+8) where g = c%2 (tensor-parallel: Wq/Wk/Wv column-sliced,
Wo row-sliced). Each core returns a partial output [S, D]; the host sums the
two head-group partials per batch.

Per-core dataflow (host pre-casts x and weights to bf16):
  x.T half-slabs arrive via XBAR transpose-DMA on one queue (concurrent
  XBAR transposes from two queues corrupt data) -> Q.T/K.T = W.T @ X.T
  (bf16), V natural with a ones column per head -> scores.T = K @ Q.T ->
  exp+mask+scale in one ACT op (bf16) -> ctx NATURAL: cx[q,65] +=
  ex-slice.T @ V'h (small matmuls; softmax denominator rides in col 64;
  only the first region of each PSUM bank issues start=True because start
  zeroes the whole bank) -> per-partition normalize on DVE -> PE-transpose
  ctx -> CT -> out = CT.T @ Wo (bf16) + bo.

Scheduling: K.T projection is the only pre-attention phase; the V
projection runs as filler inside the first attention unit (its ctx is
deferred until V' lands), Q.T chunk n+1 and the out-projection of chunk
n-1 fill later units, and the exp stream on the Scalar engine (the hard
floor at ~285us/core) runs nearly gap-free.
"""

import sys

if "/opt/trn_rl_repo" not in sys.path:
    sys.path.append("/opt/trn_rl_repo")

import numpy as np

import concourse.bass as bass
import concourse.bacc as bacc
import concourse.tile as tile
from concourse import mybir
from concourse.bass import ts
from concourse.masks import make_identity

F32 = mybir.dt.float32
F32R = mybir.dt.float32r
BF16 = mybir.dt.bfloat16
I32 = mybir.dt.int32
EXP = mybir.ActivationFunctionType.Exp

P = 128


def build_nc(S=2048, D=1024, DL=512, HD=64, debug_taps=False):
    """Build the per-core Bass program. DL = local output dim (heads*HD)."""
    ST = S // P  # token tiles (16)
    KD = D // P  # contraction tiles over D (8)
    MT = DL // P  # local d-col tiles (4)
    HL = DL // HD  # local heads (8)
    HPT = P // HD  # heads per 128-partition tile (2)
    NCH = 512  # projection token-chunk
    QH = 512  # attention q-chunk (= NCH so Q-proj chunks line up)
    NQ = S // QH  # q-chunks (4)
    QT_ = QH // P  # q-tiles per chunk (4)
    OC = 512  # out-proj col chunk
    HD1 = HD + 1
    scale = float(1.0 / (np.sqrt(np.float32(HD)) + 1e-8))

    nc = bacc.Bacc("TRN2", target_bir_lowering=False, debug=False)

    xq = nc.dram_tensor("xq", [S, D], BF16, kind="ExternalInput")
    xk = nc.dram_tensor("xk", [S, D], BF16, kind="ExternalInput")
    xv = nc.dram_tensor("xv", [S, D], BF16, kind="ExternalInput")
    msk = nc.dram_tensor("msk", [P, ST], I32, kind="ExternalInput")
    # weights arrive host-pre-arranged in SBUF layout so the DMA is one
    # contiguous 8KB-per-partition transfer instead of 1KB strided pieces
    wq = nc.dram_tensor("wq", [P, KD, DL], BF16, kind="ExternalInput")
    wk = nc.dram_tensor("wk", [P, KD, DL], BF16, kind="ExternalInput")
    wv = nc.dram_tensor("wv", [P, KD, DL], BF16, kind="ExternalInput")
    wo = nc.dram_tensor("wo", [P, MT, D], BF16, kind="ExternalInput")
    bq = nc.dram_tensor("bq", [P, MT], F32, kind="ExternalInput")
    bk = nc.dram_tensor("bk", [P, MT], F32, kind="ExternalInput")
    bv = nc.dram_tensor("bv", [1, DL], F32, kind="ExternalInput")
    bo = nc.dram_tensor("bo", [1, D], F32, kind="ExternalInput")
    out = nc.dram_tensor("out", [S, D], F32, kind="ExternalOutput")
    if debug_taps:
        dqt = nc.dram_tensor("dqt", [MT, P, S], BF16, kind="ExternalOutput")
        dkt = nc.dram_tensor("dkt", [MT, P, S], BF16, kind="ExternalOutput")
        dvp = nc.dram_tensor("dvp", [ST, P, HL * (HD + 1)], BF16, kind="ExternalOutput")
        dct = nc.dram_tensor("dct", [MT, P, S], BF16, kind="ExternalOutput")

    with tile.TileContext(nc) as tc, nc.allow_low_precision("bf16 matmul operands by design"):
        with (
            tc.tile_pool(name="pers", bufs=1) as pers,
            tc.tile_pool(name="wpool", bufs=2) as wpool,
            tc.tile_pool(name="xt", bufs=2) as xt_pool,
            tc.tile_pool(name="exp", bufs=17) as ex_pool,
            tc.tile_pool(name="cn", bufs=5) as cn_pool,
            tc.tile_pool(name="osb", bufs=1) as osb_pool,
            tc.tile_pool(name="small", bufs=2) as small,
        ):
            # ---- constants ----
            ident0 = pers.tile([P, P], F32, tag="ident0")
            make_identity(nc, ident0[:])
            identb = pers.tile([P, P], BF16, tag="identb")
            nc.vector.tensor_copy(out=identb[:], in_=ident0[:])
            ones0 = pers.tile([1, P], F32, tag="ones0")
            nc.gpsimd.memset(ones0[:], 1.0)
            ones = pers.tile([1, P], F32R, tag="ones")
            nc.vector.tensor_copy(out=ones[:], in_=ones0[:])

            # tiles for masks/biases/wo: DMAs are emitted later so the sync
            # queue serves the critical x.T transposes first
            mi = pers.tile([P, ST], I32, tag="mi")
            mf = pers.tile([P, ST], F32, tag="mf")
            mb = pers.tile([P, ST], F32, tag="mb")
            bqs = pers.tile([P, MT], F32, tag="bqs")
            bks = pers.tile([P, MT], F32, tag="bks")
            bvs = pers.tile([1, DL], F32R, tag="bvs")
            bos = pers.tile([1, D], F32R, tag="bos")
            bvb = pers.tile([P, HL, HD], F32, tag="bvb")
            bob = pers.tile([P, D], F32, tag="bob")
            wos = pers.tile([P, MT, D], BF16, tag="wos")

            # persistent activation stores
            KT = [pers.tile([P, S], BF16, tag=f"kt{m}", name=f"kt{m}") for m in range(MT)]
            QT = [pers.tile([P, S], BF16, tag=f"qt{m}", name=f"qt{m}") for m in range(MT)]
            CT = [pers.tile([P, S], BF16, tag=f"ct{m}", name=f"ct{m}") for m in range(MT)]
            VP = [
                pers.tile([P, HL, HD1], BF16, tag=f"vp{t}", name=f"vp{t}")
                for t in range(ST)
            ]
            for t in range(ST):
                nc.gpsimd.memset(VP[t][:], 1.0)

            def load_w(wdram, name, eng=None):
                w = wpool.tile([P, KD, DL], BF16, tag="w", name=name)
                (eng or nc.sync).dma_start(w[:], wdram[:, :, :])
                return w

            XH = S // 2  # x.T half-slab width (1024 tokens)

            def load_xT_half(xdram, half):
                """xt[:, kk, :] = x[half-slab, kk-slice].T via XBAR DMA.

                All on the Sync queue: concurrent XBAR transposes from two
                HWDGE queues corrupt data (measured), so keep them serial.
                """
                xt = xt_pool.tile([P, KD, XH], BF16, tag="xth", name="xth")
                for kk in range(KD):
                    nc.sync.dma_start_transpose(
                        xt[:, kk, :], xdram[ts(half, XH), ts(kk, P)]
                    )
                return xt

            def proj_units(xth, wsb, bias_sb, dst_tiles, nch, acc_pool):
                """dst[m][:, nch-chunk] = (x @ w + b).T; yields at unit edges.

                xth is the half-slab holding this nch chunk (nch%2 selects the
                512-token half of it).
                """
                off = (nch % 2) * NCH if xth.shape[2] == XH else nch * NCH
                for m in range(MT):
                    acc = acc_pool.tile([P, NCH], F32, tag="acc")
                    for kk in range(KD):
                        nc.tensor.matmul(
                            acc[:],
                            lhsT=wsb[:, kk, ts(m, P)],
                            rhs=xth[:, kk, off : off + NCH],
                            start=(kk == 0),
                            stop=(kk == KD - 1),
                        )
                    nc.vector.tensor_scalar_add(
                        dst_tiles[m][:, ts(nch, NCH)], acc[:], bias_sb[:, m : m + 1]
                    )
                    yield

            def proj_T(xth, wsb, bias_sb, dst_tiles, nch, acc_pool):
                for _ in proj_units(xth, wsb, bias_sb, dst_tiles, nch, acc_pool):
                    pass

            def vproj_units(xths, wsb, acc_pool):
                """VP[t][:, h, 0:HD] = (xv @ wv + bv)[t-tile, h-slice]."""
                for t in range(ST):
                    xth = xths[t // (XH // P)]
                    i = t % (XH // P)
                    acc = acc_pool.tile([P, DL], F32, tag="acc")
                    for kk in range(KD):
                        nc.tensor.matmul(
                            acc[:],
                            lhsT=xth[:, kk, ts(i, P)],
                            rhs=wsb[:, kk, :],
                            start=(kk == 0),
                            stop=(kk == KD - 1),
                        )
                    nc.vector.tensor_add(VP[t][:, :, 0:HD], acc[:], bvb[:])
                    yield

            def attention(
                qq, sc_pool, cx_pool, tp_pool, filler=None, pump_every=8,
                defer_first_hp=False,
            ):
                """One q-chunk (QH cols) for all head-pairs; natural-layout ctx.

                defer_first_hp: run hp=0's score/exp loop without ctx matmuls
                (collecting its ex tiles), then emit the ctx batch after the
                filler (e.g. the V projection) has been fully pumped.
                """
                it = 0
                col0 = qq * QH

                def ctx_mms(cxb, hp, kt, ex):
                    for u in range(HPT):
                        h = hp * HPT + u
                        for qt in range(QT_):
                            # start=True zeroes the WHOLE psum bank, so only
                            # the first region of each cxb bank may issue it;
                            # later regions accumulate onto the zeroed bank.
                            nc.tensor.matmul(
                                cxb[u][:, qt, 0:HD1],
                                lhsT=ex[:, u * QH + qt * P : u * QH + (qt + 1) * P],
                                rhs=VP[kt][:, h, :],
                                start=(kt == 0 and qt == 0),
                                stop=(kt == ST - 1),
                                skip_group_check=True,
                            )

                for hp in range(HL // HPT):
                    defer = defer_first_hp and hp == 0
                    exs = []
                    cxb = None
                    if not defer:
                        cxb = [
                            cx_pool.tile([P, QT_, P], F32, tag="cx", name="cx")
                            for _ in range(HPT)
                        ]
                    for kt in range(ST):
                        sc = sc_pool.tile([P, HPT * QH], F32, tag="sc")
                        for u in range(HPT):
                            mo = u * HD
                            nc.tensor.matmul(
                                sc[:, ts(u, QH)],
                                lhsT=KT[hp][mo : mo + HD, ts(kt, P)],
                                rhs=QT[hp][mo : mo + HD, col0 : col0 + QH],
                                start=True,
                                stop=True,
                            )
                        ex = ex_pool.tile([P, HPT * QH], BF16, tag="ex")
                        nc.scalar.activation(
                            ex[:], sc[:], EXP, bias=mb[:, kt : kt + 1], scale=scale
                        )
                        if defer:
                            exs.append(ex)
                        else:
                            ctx_mms(cxb, hp, kt, ex)
                        it += 1
                        if filler is not None and it % pump_every == 0:
                            next(filler, None)
                    if defer:
                        # drain the remaining filler (V projection) before the
                        # deferred ctx batch that depends on it
                        if filler is not None:
                            for _ in filler:
                                pass
                        cxb = [
                            cx_pool.tile([P, QT_, P], F32, tag="cx", name="cx")
                            for _ in range(HPT)
                        ]
                        for kt in range(ST):
                            ctx_mms(cxb, hp, kt, exs[kt])
                    # normalize: denominator is col HD of each cx region
                    recs = []
                    for u in range(HPT):
                        rec = small.tile([P, QT_], F32, tag="rec", name="rec", bufs=4)
                        nc.vector.reciprocal(rec[:], cxb[u][:, :, HD : HD + 1])
                        recs.append(rec)
                    cns = [
                        cn_pool.tile([P, HPT, HD], BF16, tag="cn", name="cn")
                        for _ in range(QT_)
                    ]
                    for qt in range(QT_):
                        for u in range(HPT):
                            nc.vector.tensor_scalar_mul(
                                cns[qt][:, u, :],
                                cxb[u][:, qt, 0:HD],
                                recs[u][:, qt : qt + 1],
                            )
                    # transpose CN -> CT[hp]
                    tp4 = tp_pool.tile([P, 4, P], BF16, tag="tpb", name="tp4")
                    for qt in range(QT_):
                        nc.tensor.transpose(tp4[:, qt, :], cns[qt][:], identb[:])
                    nc.vector.tensor_copy(out=CT[hp][:, col0 : col0 + QH], in_=tp4[:])
                    if filler is not None:
                        next(filler, None)

            def outproj_units(qq, acc_pool):
                t0 = qq * QT_
                for t in range(t0, t0 + QT_):
                    for c in range(D // OC):
                        po = acc_pool.tile([P, OC], F32, tag="acc", name="po")
                        for dd in range(MT):
                            nc.tensor.matmul(
                                po[:],
                                lhsT=CT[dd][:, ts(t, P)],
                                rhs=wos[:, dd, ts(c, OC)],
                                start=(dd == 0),
                                stop=(dd == MT - 1),
                            )
                        osb = osb_pool.tile([P, OC], F32, tag="osb")
                        nc.vector.tensor_add(osb[:], po[:], bob[:, ts(c, OC)])
                        nc.sync.dma_start(out[ts(t, P), ts(c, OC)], osb[:])
                        yield

            def outproj(qq, acc_pool):
                for _ in outproj_units(qq, acc_pool):
                    pass

            # ---- phase 1: K.T (prerequisite of attention) ----
            # K's x.T is built with PE transposes (PE/DVE are idle here) fed
            # by regular DMAs on the scalar queue, so the XBAR queue starts
            # on Q half-0 immediately.
            with (
                tc.tile_pool(name="ps1acc", bufs=6, space="PSUM") as ps1acc,
                tc.tile_pool(name="ps1tp", bufs=2, space="PSUM") as ps1tp,
            ):
                xqh0 = load_xT_half(xq, 0)
                wks = load_w(wk, "wk", eng=nc.scalar)
                nc.sync.dma_start(mi[:], msk[:, :])
                nc.vector.tensor_copy(out=mf[:], in_=mi[:])
                nc.vector.tensor_scalar_mul(mb[:], mf[:], -1.0e9)
                nc.sync.dma_start(bqs[:], bq[:, :])
                nc.sync.dma_start(bks[:], bk[:, :])
                for hh in range(2):
                    xkT = xt_pool.tile(
                        [P, KD, XH], BF16, tag="xkf", name="xkf", bufs=1
                    )
                    for i in range(XH // P):
                        t = hh * (XH // P) + i
                        xkn = xt_pool.tile(
                            [P, D], BF16, tag="xkn", name="xkn", bufs=2
                        )
                        nc.scalar.dma_start(xkn[:], xk[ts(t, P), :])
                        for half in range(2):
                            tp4 = ps1tp.tile(
                                [P, 4, P], BF16, tag="tp4", name="tp4"
                            )
                            for kk in range(4 * half, 4 * half + 4):
                                nc.tensor.transpose(
                                    tp4[:, kk % 4, :], xkn[:, ts(kk, P)], identb[:]
                                )
                            nc.vector.tensor_copy(
                                out=xkT[:, 4 * half : 4 * half + 4, ts(i, P)],
                                in_=tp4[:],
                            )
                        if i % 4 == 3:
                            proj_T(xkT, wks, bks, KT, t // 4, ps1acc)

            # ---- phase 2: Q.T chunks, attention, out-proj ----
            # Sync-queue order: wq, xq half0 (so attention can start), wv,
            # xv halves (V projection runs as filler inside the first
            # attention unit, whose ctx is deferred), xq half1.
            with (
                tc.tile_pool(name="ps2acc", bufs=1, space="PSUM") as ps2acc,
                tc.tile_pool(name="ps2sc", bufs=2, space="PSUM") as ps2sc,
                tc.tile_pool(name="ps2cx", bufs=2, space="PSUM") as ps2cx,
            ):
                from itertools import chain

                xqh0 = load_xT_half(xq, 0)
                wqs = load_w(wq, "wq")
                wvs = load_w(wv, "wv")
                bvstg = small.tile([1, D], F32, tag="bstg", name="bvstg")
                nc.sync.dma_start(bvstg[0:1, 0:DL], bv[:, :])
                nc.vector.tensor_copy(out=bvs[:], in_=bvstg[0:1, 0:DL])
                bostg = small.tile([1, D], F32, tag="bstg", name="bostg")
                nc.sync.dma_start(bostg[:], bo[:, :])
                nc.vector.tensor_copy(out=bos[:], in_=bostg[:])
                xvh = [load_xT_half(xv, h) for h in range(2)]
                nc.sync.dma_start(wos[:], wo[:, :, :])
                xqh1 = load_xT_half(xq, 1)
                xqh = [xqh0, xqh1]
                proj_T(xqh0, wqs, bqs, QT, 0, ps2acc)
                # bias row-broadcasts (needed by vproj / outproj fillers)
                bp = ps2acc.tile([P, DL], F32, tag="acc", name="bp2")
                nc.tensor.matmul(
                    bp[:], lhsT=ones[0:1, 0:P], rhs=bvs[0:1, :],
                    start=True, stop=True,
                )
                nc.vector.tensor_copy(out=bvb[:], in_=bp[:])
                for c in range(D // OC):
                    bp = ps2acc.tile([P, OC], F32, tag="acc", name="bp")
                    nc.tensor.matmul(
                        bp[:], lhsT=ones[0:1, 0:P], rhs=bos[0:1, ts(c, OC)],
                        start=True, stop=True,
                    )
                    nc.vector.tensor_copy(out=bob[:, ts(c, OC)], in_=bp[:])
                for qq in range(NQ):
                    fillers = []
                    n_units = 0
                    if qq + 1 < NQ:
                        fillers.append(
                            proj_units(
                                xqh[(qq + 1) // 2], wqs, bqs, QT, qq + 1, ps2acc
                            )
                        )
                        n_units += MT
                    if qq == 0:
                        fillers.append(vproj_units(xvh, wvs, ps2acc))
                        n_units += ST
                    if qq >= 1:
                        fillers.append(outproj_units(qq - 1, ps2acc))
                        n_units += QT_ * (D // OC)
                    filler = chain.from_iterable(fillers) if fillers else None
                    attention(
                        qq,
                        ps2sc,
                        ps2cx,
                        ps2acc,
                        filler,
                        pump_every=1 if qq == 0 else max(
                            1, ((HL // HPT) * ST) // max(n_units, 1)
                        ),
                        defer_first_hp=(qq == 0),
                    )
                    if filler is not None:
                        for _ in filler:
                            pass

                # tail out-proj inside phase 2: alternate the po accumulator
                # between the (now idle) sc pool and the acc pool for
                # pipelining, and split stores across both DMA queues (the
                # exp stream is finished, so the scalar queue is free).
                i = 0
                t0 = (NQ - 1) * QT_
                for t in range(t0, t0 + QT_):
                    for c in range(D // OC):
                        if i % 2 == 0:
                            big = ps2sc.tile(
                                [P, HPT * QH], F32, tag="sc", name="po2"
                            )
                            po = big[:, 0:OC]
                        else:
                            po = ps2acc.tile([P, OC], F32, tag="acc", name="po")[:]
                        for dd in range(MT):
                            nc.tensor.matmul(
                                po,
                                lhsT=CT[dd][:, ts(t, P)],
                                rhs=wos[:, dd, ts(c, OC)],
                                start=(dd == 0),
                                stop=(dd == MT - 1),
                            )
                        osb = osb_pool.tile([P, OC], F32, tag="osb")
                        nc.vector.tensor_add(osb[:], po, bob[:, ts(c, OC)])
                        eng = nc.sync if i % 2 == 0 else nc.scalar
                        eng.dma_start(out[ts(t, P), ts(c, OC)], osb[:])
                        i += 1
                if debug_taps:
                    for m in range(MT):
                        nc.sync.dma_start(dqt[m, :, :], QT[m][:])
                        nc.sync.dma_start(dkt[m, :, :], KT[m][:])
                        nc.sync.dma_start(dct[m, :, :], CT[m][:])
                    for t in range(ST):
                        nc.sync.dma_start(dvp[t, :, :], VP[t][:])

    nc.compile()
    return nc


_NC_CACHE = {}


def _get_nc(S, D, DL, HD):
    key = (S, D, DL, HD)
    if key not in _NC_CACHE:
        _NC_CACHE[key] = build_nc(S, D, DL, HD)
    return _NC_CACHE[key]


def _shard_inputs(q, k, v, mask, Wq, bq, Wk, bk, Wv, bv, Wo, bo):
    import ml_dtypes

    bf16 = ml_dtypes.bfloat16
    q, k, v = np.asarray(q), np.asarray(k), np.asarray(v)
    mask = np.asarray(mask)
    Wq, Wk, Wv, Wo = np.asarray(Wq), np.asarray(Wk), np.asarray(Wv), np.asarray(Wo)
    bq, bk, bv, bo = np.asarray(bq), np.asarray(bk), np.asarray(bv), np.asarray(bo)

    B, S, D = q.shape  # 4, 2048, 1024
    G = 2  # head-groups (tensor-parallel factor); B*G = 8 cores
    DL = D // G
    MT = DL // P
    ST = S // P

    f32 = np.float32
    qb = [np.ascontiguousarray(q[b], dtype=bf16) for b in range(B)]
    kb = [np.ascontiguousarray(k[b], dtype=bf16) for b in range(B)]
    vb = [np.ascontiguousarray(v[b], dtype=bf16) for b in range(B)]
    in_maps = []
    for c in range(B * G):
        b, g = c // G, c % G
        sl = slice(g * DL, (g + 1) * DL)
        bo_core = bo if g == 0 else np.zeros_like(bo)
        in_maps.append(
            {
                "xq": qb[b],
                "xk": kb[b],
                "xv": vb[b],
                "msk": np.ascontiguousarray(
                    mask[b, 0, 0].reshape(ST, P).T, dtype=np.int32
                ),
                # pre-arranged to the kernel's SBUF layout:
                # w[p, kk, n] = W[kk*128 + p, n]; wo[p, m, n] = Wo[m*128 + p, n]
                "wq": np.ascontiguousarray(
                    Wq[:, sl].reshape(D // P, P, DL).transpose(1, 0, 2), dtype=bf16
                ),
                "wk": np.ascontiguousarray(
                    Wk[:, sl].reshape(D // P, P, DL).transpose(1, 0, 2), dtype=bf16
                ),
                "wv": np.ascontiguousarray(
                    Wv[:, sl].reshape(D // P, P, DL).transpose(1, 0, 2), dtype=bf16
                ),
                "wo": np.ascontiguousarray(
                    Wo[sl, :].reshape(MT, P, D).transpose(1, 0, 2), dtype=bf16
                ),
                "bq": np.ascontiguousarray(bq[sl].reshape(MT, P).T, dtype=f32),
                "bk": np.ascontiguousarray(bk[sl].reshape(MT, P).T, dtype=f32),
                "bv": np.ascontiguousarray(bv[sl].reshape(1, DL), dtype=f32),
                "bo": np.ascontiguousarray(bo_core.reshape(1, D), dtype=f32),
            }
        )
    return in_maps


def kernel(q, k, v, mask, Wq, bq, Wk, bk, Wv, bv, Wo, bo):
    from concourse.bass_utils import run_bass_kernel_spmd

    q = np.asarray(q)
    B, S, D = q.shape  # 4, 2048, 1024
    G = 2
    nc = _get_nc(S, D, D // G, 64)
    in_maps = _shard_inputs(q, k, v, mask, Wq, bq, Wk, bk, Wv, bv, Wo, bo)

    res = run_bass_kernel_spmd(nc, in_maps, core_ids=list(range(B * G)))
    parts = [r["out"] for r in res.results]
    outf = np.stack([parts[b * G] + parts[b * G + 1] for b in range(B)], axis=0)
    return outf.astype(np.float32)
